# revision 2
# baseline (speedup 1.0000x reference)
"""Causal self-attention with relative position on 8 Trainium2 NeuronCores.

Sharding: data-parallel over batch (B=2) x tensor-parallel over heads
(16 heads -> 4 head-groups of 4). Core c handles batch c//4, heads
(c%4)*4..(c%4)*4+3. Host sums the 4 partial projections per batch + b_proj.

Design (cost-model driven):
  - bf16 matmul paths everywhere (x, W, q, k, v, att, y, wp); bf16 D1 skew
    scratch with rows padded to stride WP=L+128 whose pad holds -1e9, so
    masked (k>q) lanes of diagonal tiles read NEG directly -- no mask ops.
  - Srel^T tiles via xbar transpose: batched DmaTransposeAnt per (head, J)
    for the below-diagonal k-tiles (two halves for finer buffer rotation)
    plus 4 causally-trimmed transposes for the diagonal band; reads are
    issued per attention-pair for prefetch depth.
  - Diagonal trimming: S/Srel/exp/AV only touch q >= 128*w on diag tiles.
  - 2-bank PSUM pair tiles [128,2,512]: head-pair S tiles side by side with
    one joint Exp activation over both banks.
  - P' chunks rotate over FOUR psum banks (pp0, pp1 plus the y banks, which
    are idle during P' blocks) so the PSUM->SBUF copy latency never paces
    the PE.
  - Block emission: qkv lc-order (0,3,1,2) with P'(0),P'(3),P'(1),P'(2)
    between, then att(0..3) with proj trailing one J behind.
"""

import numpy as np
import ml_dtypes
from contextlib import ExitStack

import concourse.bass as bass
import concourse.tile as tile
from concourse import bacc, mybir
from concourse.bass_utils import run_bass_kernel_spmd

F32 = mybir.dt.float32
BF16 = mybir.dt.bfloat16
BF16_NP = ml_dtypes.bfloat16

B, L, D = 2, 2048, 1024
H, HS = 16, 64
HPC = 4              # heads per core
E = HPC * HS         # 256 e-columns per core
WP = L + 128         # padded D1 row stride
NEG = -1.0e9
SCALE = 1.0 / 8.0
NCORES = 8
EXP = mybir.ActivationFunctionType.Exp
IDENT = mybir.ActivationFunctionType.Identity

_CACHE = {}
TRACE = False


def _build_program():
    nc = bacc.Bacc("TRN2", target_bir_lowering=False, debug=False,
                   num_devices=NCORES)

    xt = nc.dram_tensor("xt", [D, L], BF16, kind="ExternalInput")
    wqkv = nc.dram_tensor("wqkv", [D, 3 * E], BF16, kind="ExternalInput")
    bqk = nc.dram_tensor("bqk", [128, 4], F32, kind="ExternalInput")
    bv = nc.dram_tensor("bv", [1, E], BF16, kind="ExternalInput")
    ert = nc.dram_tensor("ert", [128, L], BF16, kind="ExternalInput")
    wp = nc.dram_tensor("wp", [128, 2 * D], BF16, kind="ExternalInput")
    ident = nc.dram_tensor("ident", [128, 128], BF16, kind="ExternalInput")
    outT = nc.dram_tensor("outt", [D, L], BF16, kind="ExternalOutput")

    with tile.TileContext(nc) as tc, ExitStack() as ctx:
        consts = ctx.enter_context(tc.tile_pool(name="consts", bufs=1))
        persist = ctx.enter_context(tc.tile_pool(name="persist", bufs=1))
        x0pool = ctx.enter_context(tc.tile_pool(name="x0pool", bufs=1))
        xpool = ctx.enter_context(tc.tile_pool(name="xpool", bufs=2))
        stgpool = ctx.enter_context(tc.tile_pool(name="stgpool", bufs=4))
        srelbd = ctx.enter_context(tc.tile_pool(name="srelbd", bufs=8))
        sreldg = ctx.enter_context(tc.tile_pool(name="sreldg", bufs=20))
        attpool = ctx.enter_context(tc.tile_pool(name="attpool", bufs=6))
        outpool = ctx.enter_context(tc.tile_pool(name="outpool", bufs=4))
        normpool = ctx.enter_context(tc.tile_pool(name="normpool", bufs=2))
        drampool = ctx.enter_context(tc.tile_pool(name="dram", bufs=1,
                                                  space="DRAM"))
        ps = ctx.enter_context(tc.tile_pool(name="ps", bufs=1, space="PSUM"))

        cnt = {"cp": 0, "pp": 0}
        PPTAGS = ("pp0", "pp1", "y0", "y1", "sA", "sB")
        PROJTAGS = ("pp0", "pp1", "sA", "sB")

        def rr_copy(dst, src):
            if cnt["cp"] % 2 == 0:
                nc.scalar.copy(dst, src)
            else:
                nc.vector.tensor_copy(dst, src)
            cnt["cp"] += 1

        # ---- persistent activations ----
        qT_sb = persist.tile([128, 2, L], BF16)   # [64*hp + d, et, l]
        kT_sb = persist.tile([128, 2, L], BF16)
        y_sb = persist.tile([128, 2, L], BF16)
        v_sb = persist.tile([128, 16, HPC * 96], BF16)  # V'' per k-tile
        p_sb_stride = v_sb[:].ap[0][0]

        # ---- startup loads: x(0) dt0 slice + w dt0 first so PE starts ASAP
        x0a = x0pool.tile([128, 1, 512], BF16, tag="x0a", name="x0a")
        nc.sync.dma_start(x0a[:, 0], bass.AP(xt, 0, [[L, 128], [1, 512]]))
        w_sb = consts.tile([128, 8, 3 * E], BF16)
        _wv = wqkv.ap().rearrange("(o p) e -> p o e", p=128)
        nc.sync.dma_start(w_sb[:, 0], _wv[:, 0])
        x0b = x0pool.tile([128, 7, 512], BF16, tag="x0b", name="x0b")
        nc.sync.dma_start(
            x0b[:], bass.AP(xt, 128 * L, [[L, 128], [128 * L, 7], [1, 512]]))
        for dt_ in range(1, 8):
            nc.sync.dma_start(w_sb[:, dt_], _wv[:, dt_])

        bqk_sb = consts.tile([128, 4], F32)
        nc.sync.dma_start(bqk_sb[:], bqk.ap())
        id_sb = consts.tile([128, 128], BF16)
        nc.sync.dma_start(id_sb[:], ident.ap())
        bv_sb = consts.tile([1, E], BF16)
        nc.sync.dma_start(bv_sb[:], bv.ap())
        ones_sb = consts.tile([1, 128], BF16)
        nc.gpsimd.memset(ones_sb[:], 1.0)
        neg_sb = consts.tile([128, 128], BF16)
        nc.gpsimd.memset(neg_sb[:], NEG)

        # ones column + zero pad of V'' (cols 64..96 per head)
        for h in range(HPC):
            nc.gpsimd.memset(v_sb[:, :, 96 * h + 64:96 * h + 65], 1.0)
            nc.gpsimd.memset(v_sb[:, :, 96 * h + 65:96 * (h + 1)], 0.0)

        # D1 scratch: 4 heads x L rows x WP bf16
        d1 = drampool.tile([HPC * L * WP], BF16, name="d1")

        def late_consts_and_pads():
            ert_sb = consts.tile([128, L], BF16)
            nc.sync.dma_start(ert_sb[:], ert.ap())
            wp_sb = consts.tile([128, 2, D], BF16)
            nc.sync.dma_start(wp_sb[:], wp.ap().rearrange(
                "p (u m) -> p u m", u=2))
            for h in range(HPC):  # NEG pad cols [L, L+128) of every row
                dst = bass.AP(d1.tensor, d1.offset + h * L * WP + L,
                              [[WP, 128], [128 * WP, 16], [1, 128]])
                src = bass.AP(neg_sb.tensor, neg_sb.offset,
                              [[neg_sb[:].ap[0][0], 128], [0, 16], [1, 128]])
                nc.sync.dma_start(dst, src)
            return ert_sb, wp_sb

        # ================= phase emitters =================
        def load_x(lc):
            xl = xpool.tile([128, 8, 512], BF16, tag="x", name=f"x_{lc}")
            nc.sync.dma_start(
                xl[:], bass.AP(xt, 512 * lc,
                               [[L, 128], [128 * L, 8], [1, 512]]))
            return xl

        def qkv(lc, xl):
            def xsl(dt_):
                if lc != 0:
                    return xl[:, dt_]
                return x0a[:, 0] if dt_ == 0 else x0b[:, dt_ - 1]

            qp = ps.tile([128, 2, 512], F32, tag="sA", name=f"qp_{lc}")
            kp = ps.tile([128, 2, 512], F32, tag="sB", name=f"kp_{lc}")
            vp = [ps.tile([128, 2, E], F32, tag=f"y{u}", name=f"vp_{lc}_{u}")
                  for u in range(2)]
            for dt_ in range(8):
                for i in range(4):
                    dstp = qp if i < 2 else kp
                    nc.tensor.matmul(
                        dstp[:, i % 2],
                        w_sb[:, dt_, 128 * i:128 * (i + 1)],
                        xsl(dt_),
                        start=(dt_ == 0), stop=(dt_ == 7))
                for i in range(4):
                    # PSUM start=True zeroes the whole 2KB bank: only the
                    # first group per bank starts; its sibling accumulates
                    # onto the already-zeroed region
                    nc.tensor.matmul(
                        vp[i // 2][:, i % 2],
                        xsl(dt_)[:, 128 * i:128 * (i + 1)],
                        w_sb[:, dt_, 2 * E:3 * E],
                        start=(dt_ == 0 and i % 2 == 0), stop=False,
                        skip_group_check=(i % 2 == 1))
            for i in range(4):  # +bv via ones row (K=1)
                nc.tensor.matmul(vp[i // 2][:, i % 2], ones_sb[:], bv_sb[:],
                                 start=False, stop=True,
                                 skip_group_check=(i % 2 == 1))
            lsl = slice(512 * lc, 512 * (lc + 1))
            for i in range(4):
                dst = qT_sb[:, i % 2, lsl] if i < 2 else kT_sb[:, i % 2, lsl]
                srcp = qp if i < 2 else kp
                nc.scalar.activation(dst, srcp[:, i % 2], IDENT,
                                     bias=bqk_sb[:, i:i + 1])
            for u in range(2):
                lt = 4 * lc + 2 * u
                src = bass.AP(vp[u].tensor, vp[u].offset,
                              [[vp[u][:].ap[0][0], 128], [E, 2], [64, 4],
                               [1, 64]])
                dst = bass.AP(v_sb.tensor, v_sb.offset + lt * (HPC * 96),
                              [[p_sb_stride, 128], [HPC * 96, 2], [96, 4],
                               [1, 64]])
                nc.vector.tensor_copy(dst, src)

        def pprime(J, ert_sb):
            span = 512 * (J + 1)
            c0 = 3 - J
            nmp = 2 if J >= 2 else 4  # m-rows per staging tile (<=8KB)
            for h in range(HPC):
                et, j = h // 2, h % 2
                pr = slice(64 * j, 64 * j + 64)
                for g in range(4 // nmp):
                    stg = stgpool.tile([128, nmp, span], BF16, tag="stg",
                                       name=f"stg_{h}_{J}_{g}")
                    for mh in range(nmp):
                        mp = nmp * g + mh
                        m = 4 * J + mp
                        for C in range(c0, 4):
                            pp = ps.tile([128, 512], F32,
                                         tag=PPTAGS[cnt["pp"] % 4],
                                         name=f"pp_{h}_{J}_{mp}_{C}")
                            cnt["pp"] += 1
                            b0 = 256 if (C == c0 and mp <= 1) else 0
                            nc.tensor.matmul(
                                pp[:, b0:],
                                qT_sb[pr, et, 128 * m:128 * (m + 1)],
                                ert_sb[pr, 512 * C + b0:512 * (C + 1)],
                                start=True, stop=True)
                            rr_copy(
                                stg[:, mh,
                                    512 * (C - c0) + b0:512 * (C - c0 + 1)],
                                pp[:, b0:])
                    base = (h * L * WP + (512 * J + 128 * nmp * g) * WP
                            + 512 * c0)
                    nc.sync.dma_start(
                        bass.AP(d1.tensor, d1.offset + base,
                                [[WP, 128], [128 * WP, nmp], [1, span]]),
                        stg[:])

        srel_tiles = {}

        def srel_reads(J, pair):
            # per head: batched transposes for the below-diag k-tiles in two
            # halves (finer pool rotation -> earlier prefetch of the next
            # pair's reads), plus 4 causally-trimmed diagonal transposes
            for h in (2 * pair, 2 * pair + 1):
                base = d1.offset + h * L * WP
                bds = []
                if J > 0:
                    src0 = base + 512 * J * (WP - 1) + (L - 1)
                    for half in range(2):
                        bdh = srelbd.tile([128, 2 * J, 512], BF16, tag="bd",
                                          name=f"srbd_{h}_{J}_{half}")
                        nc.sync.dma_start_transpose(
                            bdh[:],
                            bass.AP(d1.tensor, src0 + half * 256 * J,
                                    [[WP - 1, 512], [1, 256 * J]]))
                        bds.append(bdh)
                dgs = []
                for w in range(4):
                    span = 512 - 128 * w
                    t = 4 * J + w
                    dg = sreldg.tile([128, 512], BF16, tag="dg",
                                     name=f"srdg_{h}_{J}_{w}")
                    src = bass.AP(
                        d1.tensor,
                        base + (512 * J + 128 * w) * (WP - 1) + (L - 1)
                        + 128 * t,
                        [[WP - 1, span], [1, 128]])
                    nc.sync.dma_start_transpose(dg[:, :span], src)
                    dgs.append(dg)
                srel_tiles[h] = (bds, dgs)

        def att(J):
            nt = 4 * (J + 1)
            if J == 0:
                srel_reads(0, 0)
            for et in range(2):
                sr = [srel_tiles[2 * et], srel_tiles[2 * et + 1]]
                y_ps = [ps.tile([96, 512], F32, tag=f"y{j}",
                                name=f"y_{J}_{et}_{j}") for j in range(2)]
                att_prev, pq0 = None, 0
                for t in range(nt):
                    w = t - 4 * J
                    q0 = 128 * w if w > 0 else 0
                    sp = ps.tile([128, 2, 512], F32,
                                 tag=("sA" if t % 2 == 0 else "sB"),
                                 name=f"s_{J}_{et}_{t}")
                    for j in range(2):
                        pr = slice(64 * j, 64 * j + 64)
                        nc.tensor.matmul(
                            sp[:, j, q0:],
                            kT_sb[pr, et, 128 * t:128 * (t + 1)],
                            qT_sb[pr, et, 512 * J + q0:512 * (J + 1)],
                            start=True, stop=False)
                    for j in range(2):
                        bds, dgs = sr[j]
                        rhs = (dgs[w][:, :512 - q0] if w >= 0
                               else bds[t // (2 * J)][:, t % (2 * J), :])
                        nc.tensor.matmul(sp[:, j, q0:], id_sb[:], rhs,
                                         start=False, stop=True)
                    att_t = attpool.tile([128, 2, 512], BF16, tag="att",
                                         name=f"att_{J}_{et}_{t}")
                    nc.scalar.activation(att_t[:, :, q0:], sp[:, :, q0:],
                                         EXP, scale=SCALE)
                    if t > 0:
                        for j in range(2):
                            h = 2 * et + j
                            nc.tensor.matmul(
                                y_ps[j][:, pq0:],
                                v_sb[:, t - 1, 96 * h:96 * (h + 1)],
                                att_prev[:, j, pq0:],
                                start=(t == 1), stop=False)
                    att_prev, pq0 = att_t, q0
                    if et == 0 and t == min(1, nt - 1):
                        srel_reads(J, 1)
                    if et == 1 and J < 3 and t == max(0, nt // 3 - 1):
                        srel_reads(J + 1, 0)
                for j in range(2):
                    h = 2 * et + j
                    nc.tensor.matmul(
                        y_ps[j][:, pq0:],
                        v_sb[:, nt - 1, 96 * h:96 * (h + 1)],
                        att_prev[:, j, pq0:],
                        start=(nt == 1), stop=True)
                for j in range(2):
                    recip = normpool.tile([1, 512], F32, tag="recip",
                                          name=f"rc_{J}_{et}_{j}")
                    nc.vector.reciprocal(recip[:], y_ps[j][64:65, :])
                    rb = normpool.tile([64, 512], F32, tag="rb",
                                       name=f"rb_{J}_{et}_{j}")
                    nc.gpsimd.partition_broadcast(rb[:], recip[:], channels=64)
                    nc.vector.tensor_tensor(
                        y_sb[64 * j:64 * j + 64, et, 512 * J:512 * (J + 1)],
                        y_ps[j][0:64, :], rb[:], mybir.AluOpType.mult)

        def proj(J, wp_sb):
            for dt_ in range(8):
                pr = ps.tile([128, 512], F32, tag=PROJTAGS[dt_ % 2],
                             name=f"proj_{J}_{dt_}")
                for et in range(2):
                    nc.tensor.matmul(
                        pr[:],
                        wp_sb[:, et, 128 * dt_:128 * (dt_ + 1)],
                        y_sb[:, et, 512 * J:512 * (J + 1)],
                        start=(et == 0), stop=(et == 1))
                o_t = outpool.tile([128, 512], BF16, tag="o",
                                   name=f"o_{J}_{dt_}")
                rr_copy(o_t[:], pr[:])
                nc.sync.dma_start(
                    outT.ap()[128 * dt_:128 * (dt_ + 1),
                              512 * J:512 * (J + 1)], o_t[:])

        # ================= emission schedule (block order) =================
        qkv(0, None)
        xl3 = load_x(3)
        ert_sb, wp_sb = late_consts_and_pads()
        pprime(0, ert_sb)
        xl1 = load_x(1)
        qkv(3, xl3)
        pprime(3, ert_sb)
        xl2 = load_x(2)
        qkv(1, xl1)
        pprime(1, ert_sb)
        qkv(2, xl2)
        pprime(2, ert_sb)
        att(0)
        att(1)
        proj(0, wp_sb)
        att(2)
        proj(1, wp_sb)
        att(3)
        proj(2, wp_sb)
        proj(3, wp_sb)

    nc.compile()
    return nc


def kernel(x, W_attn, b_attn, W_proj, b_proj, Er):
    x = np.ascontiguousarray(x, dtype=np.float32)
    W_attn = np.ascontiguousarray(W_attn, dtype=np.float32)
    b_attn = np.ascontiguousarray(b_attn, dtype=np.float32)
    W_proj = np.ascontiguousarray(W_proj, dtype=np.float32)
    b_proj = np.ascontiguousarray(b_proj, dtype=np.float32)
    Er = np.ascontiguousarray(Er, dtype=np.float32)

    if "nc" not in _CACHE:
        _CACHE["nc"] = _build_program()
    nc = _CACHE["nc"]

    ident = np.eye(128, dtype=BF16_NP)
    ert_full = Er[-L:, :].T                         # [64, L]
    ert2 = np.concatenate([ert_full, ert_full], axis=0).astype(BF16_NP)

    in_maps = []
    for c in range(NCORES):
        b, hg = divmod(c, 4)
        e0 = hg * E
        cols = np.r_[e0:e0 + E, D + e0:D + e0 + E, 2 * D + e0:2 * D + e0 + E]
        wqkv_ = W_attn[:, cols].astype(BF16_NP)          # [D, 768]
        bq = b_attn[e0:e0 + E]
        bk = b_attn[D + e0:D + e0 + E]
        bv_ = b_attn[2 * D + e0:2 * D + e0 + E]
        bqk_ = np.concatenate([bq, bk]).reshape(4, 128).T.copy()  # [128, 4]
        wp_ = W_proj[e0:e0 + E, :].reshape(2, 128, D).transpose(
            1, 0, 2).reshape(128, 2 * D).astype(BF16_NP)
        in_maps.append({
            "xt": x[b].T.astype(BF16_NP),
            "wqkv": wqkv_,
            "bqk": bqk_,
            "bv": bv_.reshape(1, E).astype(BF16_NP),
            "ert": ert2,
            "wp": wp_,
            "ident": ident,
        })

    res = run_bass_kernel_spmd(nc, in_maps, core_ids=list(range(NCORES)),
                               trace=TRACE)
    _CACHE["last_results"] = res

    out = np.zeros((B, L, D), dtype=np.float32)
    for c in range(NCORES):
        out[c // 4] += res.results[c]["outt"].T.astype(np.float32)
    out += b_proj[None, None, :]
    return out


# revision 3
# speedup vs baseline: 1.0083x; 1.0083x over previous
"""Causal self-attention with relative position on 8 Trainium2 NeuronCores.

Sharding: data-parallel over batch (B=2) x tensor-parallel over heads
(16 heads -> 4 head-groups of 4). Core c handles batch c//4, heads
(c%4)*4..(c%4)*4+3. Host sums the 4 partial projections per batch + b_proj.

Design (cost-model driven):
  - bf16 matmul paths everywhere (x, W, q, k, v, att, y, wp); bf16 D1 skew
    scratch with rows padded to stride WP=L+128 whose pad holds -1e9, so
    masked (k>q) lanes of diagonal tiles read NEG directly -- no mask ops.
  - Srel^T tiles via xbar transpose: batched DmaTransposeAnt per (head, J)
    for the below-diagonal k-tiles (two halves for finer buffer rotation)
    plus 4 causally-trimmed transposes for the diagonal band; reads are
    issued per attention-pair for prefetch depth.
  - Diagonal trimming: S/Srel/exp/AV only touch q >= 128*w on diag tiles.
  - 2-bank PSUM pair tiles [128,2,512]: head-pair S tiles side by side with
    one joint Exp activation over both banks.
  - P' chunks rotate over FOUR psum banks (pp0, pp1 plus the y banks, which
    are idle during P' blocks) so the PSUM->SBUF copy latency never paces
    the PE.
  - Block emission: qkv lc-order (0,3,1,2) with P'(0),P'(3),P'(1),P'(2)
    between, then att(0..3) with proj trailing one J behind.
"""

import numpy as np
import ml_dtypes
from contextlib import ExitStack

import concourse.bass as bass
import concourse.tile as tile
from concourse import bacc, mybir
from concourse.bass_utils import run_bass_kernel_spmd

F32 = mybir.dt.float32
BF16 = mybir.dt.bfloat16
BF16_NP = ml_dtypes.bfloat16

B, L, D = 2, 2048, 1024
H, HS = 16, 64
HPC = 4              # heads per core
E = HPC * HS         # 256 e-columns per core
WP = L + 128         # padded D1 row stride
NEG = -1.0e9
SCALE = 1.0 / 8.0
NCORES = 8
EXP = mybir.ActivationFunctionType.Exp
IDENT = mybir.ActivationFunctionType.Identity

_CACHE = {}
TRACE = False


def _build_program():
    nc = bacc.Bacc("TRN2", target_bir_lowering=False, debug=False,
                   num_devices=NCORES)

    xt = nc.dram_tensor("xt", [D, L], BF16, kind="ExternalInput")
    wqkv = nc.dram_tensor("wqkv", [D, 3 * E], BF16, kind="ExternalInput")
    bqk = nc.dram_tensor("bqk", [128, 4], F32, kind="ExternalInput")
    bv = nc.dram_tensor("bv", [1, E], BF16, kind="ExternalInput")
    ert = nc.dram_tensor("ert", [128, L], BF16, kind="ExternalInput")
    wp = nc.dram_tensor("wp", [128, 2 * D], BF16, kind="ExternalInput")
    ident = nc.dram_tensor("ident", [128, 128], BF16, kind="ExternalInput")
    outT = nc.dram_tensor("outt", [D, L], BF16, kind="ExternalOutput")

    with tile.TileContext(nc) as tc, ExitStack() as ctx:
        consts = ctx.enter_context(tc.tile_pool(name="consts", bufs=1))
        persist = ctx.enter_context(tc.tile_pool(name="persist", bufs=1))
        x0pool = ctx.enter_context(tc.tile_pool(name="x0pool", bufs=1))
        xpool = ctx.enter_context(tc.tile_pool(name="xpool", bufs=2))
        stgpool = ctx.enter_context(tc.tile_pool(name="stgpool", bufs=4))
        srelbd = ctx.enter_context(tc.tile_pool(name="srelbd", bufs=8))
        sreldg = ctx.enter_context(tc.tile_pool(name="sreldg", bufs=20))
        attpool = ctx.enter_context(tc.tile_pool(name="attpool", bufs=6))
        outpool = ctx.enter_context(tc.tile_pool(name="outpool", bufs=4))
        normpool = ctx.enter_context(tc.tile_pool(name="normpool", bufs=2))
        drampool = ctx.enter_context(tc.tile_pool(name="dram", bufs=1,
                                                  space="DRAM"))
        ps = ctx.enter_context(tc.tile_pool(name="ps", bufs=1, space="PSUM"))

        cnt = {"cp": 0, "pp": 0}
        PPTAGS = ("pp0", "pp1", "y0", "y1", "sA", "sB")
        PROJTAGS = ("pp0", "pp1", "sA", "sB")

        def rr_copy(dst, src):
            if cnt["cp"] % 2 == 0:
                nc.scalar.copy(dst, src)
            else:
                nc.vector.tensor_copy(dst, src)
            cnt["cp"] += 1

        # ---- persistent activations ----
        qT_sb = persist.tile([128, 2, L], BF16)   # [64*hp + d, et, l]
        kT_sb = persist.tile([128, 2, L], BF16)
        y_sb = persist.tile([128, 2, L], BF16)
        v_sb = persist.tile([128, 16, HPC * 96], BF16)  # V'' per k-tile
        p_sb_stride = v_sb[:].ap[0][0]

        # ---- startup loads: x(0) dt0 slice + w dt0 first so PE starts ASAP
        x0a = x0pool.tile([128, 1, 512], BF16, tag="x0a", name="x0a")
        nc.sync.dma_start(x0a[:, 0], bass.AP(xt, 0, [[L, 128], [1, 512]]))
        w_sb = consts.tile([128, 8, 3 * E], BF16)
        _wv = wqkv.ap().rearrange("(o p) e -> p o e", p=128)
        nc.sync.dma_start(w_sb[:, 0], _wv[:, 0])
        x0b = x0pool.tile([128, 7, 512], BF16, tag="x0b", name="x0b")
        nc.sync.dma_start(
            x0b[:, 0:2],
            bass.AP(xt, 128 * L, [[L, 128], [128 * L, 2], [1, 512]]))
        nc.sync.dma_start(
            x0b[:, 2:7],
            bass.AP(xt, 3 * 128 * L, [[L, 128], [128 * L, 5], [1, 512]]))
        for dt_ in range(1, 8):
            nc.sync.dma_start(w_sb[:, dt_], _wv[:, dt_])

        bqk_sb = consts.tile([128, 4], F32)
        nc.sync.dma_start(bqk_sb[:], bqk.ap())
        id_sb = consts.tile([128, 128], BF16)
        nc.sync.dma_start(id_sb[:], ident.ap())
        bv_sb = consts.tile([1, E], BF16)
        nc.sync.dma_start(bv_sb[:], bv.ap())
        ones_sb = consts.tile([1, 128], BF16)
        nc.gpsimd.memset(ones_sb[:], 1.0)
        warm_sb = consts.tile([1, 512], BF16)
        nc.vector.memset(warm_sb[:], 0.0)
        # warm-up matmuls: keep the PE busy (and its p-state ramping) while
        # the first x/w DMAs land
        for wi in range(6):
            wt = ps.tile([128, 512], F32, tag=f"pp{wi % 2}",
                         name=f"warm_{wi}")
            nc.tensor.matmul(wt[:], ones_sb[:], warm_sb[:],
                             start=True, stop=True)
        neg_sb = consts.tile([128, 128], BF16)
        nc.gpsimd.memset(neg_sb[:], NEG)

        # ones column + zero pad of V'' (cols 64..96 per head)
        for h in range(HPC):
            nc.gpsimd.memset(v_sb[:, :, 96 * h + 64:96 * h + 65], 1.0)
            nc.gpsimd.memset(v_sb[:, :, 96 * h + 65:96 * (h + 1)], 0.0)

        # D1 scratch: 4 heads x L rows x WP bf16
        d1 = drampool.tile([HPC * L * WP], BF16, name="d1")

        def late_consts_and_pads():
            ert_sb = consts.tile([128, L], BF16)
            nc.sync.dma_start(ert_sb[:], ert.ap())
            wp_sb = consts.tile([128, 2, D], BF16)
            nc.sync.dma_start(wp_sb[:], wp.ap().rearrange(
                "p (u m) -> p u m", u=2))
            for h in range(HPC):  # NEG pad cols [L, L+128) of every row
                dst = bass.AP(d1.tensor, d1.offset + h * L * WP + L,
                              [[WP, 128], [128 * WP, 16], [1, 128]])
                src = bass.AP(neg_sb.tensor, neg_sb.offset,
                              [[neg_sb[:].ap[0][0], 128], [0, 16], [1, 128]])
                nc.sync.dma_start(dst, src)
            return ert_sb, wp_sb

        # ================= phase emitters =================
        def load_x(lc):
            xl = xpool.tile([128, 8, 512], BF16, tag="x", name=f"x_{lc}")
            nc.sync.dma_start(
                xl[:], bass.AP(xt, 512 * lc,
                               [[L, 128], [128 * L, 8], [1, 512]]))
            return xl

        def qkv(lc, xl):
            def xsl(dt_):
                if lc != 0:
                    return xl[:, dt_]
                return x0a[:, 0] if dt_ == 0 else x0b[:, dt_ - 1]

            qp = ps.tile([128, 2, 512], F32, tag="sA", name=f"qp_{lc}")
            kp = ps.tile([128, 2, 512], F32, tag="sB", name=f"kp_{lc}")
            vp = [ps.tile([128, 2, E], F32, tag=f"y{u}", name=f"vp_{lc}_{u}")
                  for u in range(2)]
            for dt_ in range(8):
                for i in range(4):
                    dstp = qp if i < 2 else kp
                    nc.tensor.matmul(
                        dstp[:, i % 2],
                        w_sb[:, dt_, 128 * i:128 * (i + 1)],
                        xsl(dt_),
                        start=(dt_ == 0), stop=(dt_ == 7))
                for i in range(4):
                    # PSUM start=True zeroes the whole 2KB bank: only the
                    # first group per bank starts; its sibling accumulates
                    # onto the already-zeroed region
                    nc.tensor.matmul(
                        vp[i // 2][:, i % 2],
                        xsl(dt_)[:, 128 * i:128 * (i + 1)],
                        w_sb[:, dt_, 2 * E:3 * E],
                        start=(dt_ == 0 and i % 2 == 0), stop=False,
                        skip_group_check=(i % 2 == 1))
            for i in range(4):  # +bv via ones row (K=1)
                nc.tensor.matmul(vp[i // 2][:, i % 2], ones_sb[:], bv_sb[:],
                                 start=False, stop=True,
                                 skip_group_check=(i % 2 == 1))
            lsl = slice(512 * lc, 512 * (lc + 1))
            for i in range(4):
                dst = qT_sb[:, i % 2, lsl] if i < 2 else kT_sb[:, i % 2, lsl]
                srcp = qp if i < 2 else kp
                nc.scalar.activation(dst, srcp[:, i % 2], IDENT,
                                     bias=bqk_sb[:, i:i + 1])
            for u in range(2):
                lt = 4 * lc + 2 * u
                src = bass.AP(vp[u].tensor, vp[u].offset,
                              [[vp[u][:].ap[0][0], 128], [E, 2], [64, 4],
                               [1, 64]])
                dst = bass.AP(v_sb.tensor, v_sb.offset + lt * (HPC * 96),
                              [[p_sb_stride, 128], [HPC * 96, 2], [96, 4],
                               [1, 64]])
                nc.vector.tensor_copy(dst, src)

        def pprime(J, ert_sb):
            span = 512 * (J + 1)
            c0 = 3 - J
            nmp = 2 if J >= 2 else 4  # m-rows per staging tile (<=8KB)
            for h in range(HPC):
                et, j = h // 2, h % 2
                pr = slice(64 * j, 64 * j + 64)
                for g in range(4 // nmp):
                    stg = stgpool.tile([128, nmp, span], BF16, tag="stg",
                                       name=f"stg_{h}_{J}_{g}")
                    for mh in range(nmp):
                        mp = nmp * g + mh
                        m = 4 * J + mp
                        for C in range(c0, 4):
                            pp = ps.tile([128, 512], F32,
                                         tag=PPTAGS[cnt["pp"] % 4],
                                         name=f"pp_{h}_{J}_{mp}_{C}")
                            cnt["pp"] += 1
                            b0 = 256 if (C == c0 and mp <= 1) else 0
                            nc.tensor.matmul(
                                pp[:, b0:],
                                qT_sb[pr, et, 128 * m:128 * (m + 1)],
                                ert_sb[pr, 512 * C + b0:512 * (C + 1)],
                                start=True, stop=True)
                            rr_copy(
                                stg[:, mh,
                                    512 * (C - c0) + b0:512 * (C - c0 + 1)],
                                pp[:, b0:])
                    base = (h * L * WP + (512 * J + 128 * nmp * g) * WP
                            + 512 * c0)
                    nc.sync.dma_start(
                        bass.AP(d1.tensor, d1.offset + base,
                                [[WP, 128], [128 * WP, nmp], [1, span]]),
                        stg[:])

        srel_tiles = {}

        def srel_reads(J, pair):
            # per head: batched transposes for the below-diag k-tiles in two
            # halves (finer pool rotation -> earlier prefetch of the next
            # pair's reads), plus 4 causally-trimmed diagonal transposes
            for h in (2 * pair, 2 * pair + 1):
                base = d1.offset + h * L * WP
                bds = []
                if J > 0:
                    src0 = base + 512 * J * (WP - 1) + (L - 1)
                    for half in range(2):
                        bdh = srelbd.tile([128, 2 * J, 512], BF16, tag="bd",
                                          name=f"srbd_{h}_{J}_{half}")
                        nc.sync.dma_start_transpose(
                            bdh[:],
                            bass.AP(d1.tensor, src0 + half * 256 * J,
                                    [[WP - 1, 512], [1, 256 * J]]))
                        bds.append(bdh)
                dgs = []
                for w in range(4):
                    span = 512 - 128 * w
                    t = 4 * J + w
                    dg = sreldg.tile([128, 512], BF16, tag="dg",
                                     name=f"srdg_{h}_{J}_{w}")
                    src = bass.AP(
                        d1.tensor,
                        base + (512 * J + 128 * w) * (WP - 1) + (L - 1)
                        + 128 * t,
                        [[WP - 1, span], [1, 128]])
                    nc.sync.dma_start_transpose(dg[:, :span], src)
                    dgs.append(dg)
                srel_tiles[h] = (bds, dgs)

        def att(J):
            nt = 4 * (J + 1)
            for et in range(2):
                sr = [srel_tiles[2 * et], srel_tiles[2 * et + 1]]
                y_ps = [ps.tile([96, 512], F32, tag=f"y{j}",
                                name=f"y_{J}_{et}_{j}") for j in range(2)]
                att_prev, pq0 = None, 0
                for t in range(nt):
                    w = t - 4 * J
                    q0 = 128 * w if w > 0 else 0
                    sp = ps.tile([128, 2, 512], F32,
                                 tag=("sA" if t % 2 == 0 else "sB"),
                                 name=f"s_{J}_{et}_{t}")
                    for j in range(2):
                        pr = slice(64 * j, 64 * j + 64)
                        nc.tensor.matmul(
                            sp[:, j, q0:],
                            kT_sb[pr, et, 128 * t:128 * (t + 1)],
                            qT_sb[pr, et, 512 * J + q0:512 * (J + 1)],
                            start=True, stop=False)
                    for j in range(2):
                        bds, dgs = sr[j]
                        rhs = (dgs[w][:, :512 - q0] if w >= 0
                               else bds[t // (2 * J)][:, t % (2 * J), :])
                        nc.tensor.matmul(sp[:, j, q0:], id_sb[:], rhs,
                                         start=False, stop=True)
                    att_t = attpool.tile([128, 2, 512], BF16, tag="att",
                                         name=f"att_{J}_{et}_{t}")
                    nc.scalar.activation(att_t[:, :, q0:], sp[:, :, q0:],
                                         EXP, scale=SCALE)
                    if t > 0:
                        for j in range(2):
                            h = 2 * et + j
                            nc.tensor.matmul(
                                y_ps[j][:, pq0:],
                                v_sb[:, t - 1, 96 * h:96 * (h + 1)],
                                att_prev[:, j, pq0:],
                                start=(t == 1), stop=False)
                    att_prev, pq0 = att_t, q0
                    if et == 0 and t == 0:
                        srel_reads(J, 1)
                    if et == 1 and J < 3 and t == max(0, nt // 3 - 1):
                        srel_reads(J + 1, 0)
                for j in range(2):
                    h = 2 * et + j
                    nc.tensor.matmul(
                        y_ps[j][:, pq0:],
                        v_sb[:, nt - 1, 96 * h:96 * (h + 1)],
                        att_prev[:, j, pq0:],
                        start=(nt == 1), stop=True)
                for j in range(2):
                    recip = normpool.tile([1, 512], F32, tag="recip",
                                          name=f"rc_{J}_{et}_{j}")
                    nc.vector.reciprocal(recip[:], y_ps[j][64:65, :])
                    rb = normpool.tile([64, 512], F32, tag="rb",
                                       name=f"rb_{J}_{et}_{j}")
                    nc.gpsimd.partition_broadcast(rb[:], recip[:], channels=64)
                    nc.vector.tensor_tensor(
                        y_sb[64 * j:64 * j + 64, et, 512 * J:512 * (J + 1)],
                        y_ps[j][0:64, :], rb[:], mybir.AluOpType.mult)

        def proj(J, wp_sb):
            for dt_ in range(8):
                pr = ps.tile([128, 512], F32, tag=PROJTAGS[dt_ % 2],
                             name=f"proj_{J}_{dt_}")
                for et in range(2):
                    nc.tensor.matmul(
                        pr[:],
                        wp_sb[:, et, 128 * dt_:128 * (dt_ + 1)],
                        y_sb[:, et, 512 * J:512 * (J + 1)],
                        start=(et == 0), stop=(et == 1))
                o_t = outpool.tile([128, 512], BF16, tag="o",
                                   name=f"o_{J}_{dt_}")
                rr_copy(o_t[:], pr[:])
                nc.sync.dma_start(
                    outT.ap()[128 * dt_:128 * (dt_ + 1),
                              512 * J:512 * (J + 1)], o_t[:])

        # ================= emission schedule (block order) =================
        qkv(0, None)
        xl3 = load_x(3)
        ert_sb, wp_sb = late_consts_and_pads()
        pprime(0, ert_sb)
        xl1 = load_x(1)
        qkv(3, xl3)
        pprime(3, ert_sb)
        xl2 = load_x(2)
        qkv(1, xl1)
        pprime(1, ert_sb)
        srel_reads(0, 0)
        qkv(2, xl2)
        pprime(2, ert_sb)
        att(0)
        att(1)
        proj(0, wp_sb)
        att(2)
        proj(1, wp_sb)
        att(3)
        proj(2, wp_sb)
        proj(3, wp_sb)

    nc.compile()
    return nc


def kernel(x, W_attn, b_attn, W_proj, b_proj, Er):
    x = np.ascontiguousarray(x, dtype=np.float32)
    W_attn = np.ascontiguousarray(W_attn, dtype=np.float32)
    b_attn = np.ascontiguousarray(b_attn, dtype=np.float32)
    W_proj = np.ascontiguousarray(W_proj, dtype=np.float32)
    b_proj = np.ascontiguousarray(b_proj, dtype=np.float32)
    Er = np.ascontiguousarray(Er, dtype=np.float32)

    if "nc" not in _CACHE:
        _CACHE["nc"] = _build_program()
    nc = _CACHE["nc"]

    ident = np.eye(128, dtype=BF16_NP)
    ert_full = Er[-L:, :].T                         # [64, L]
    ert2 = np.concatenate([ert_full, ert_full], axis=0).astype(BF16_NP)

    in_maps = []
    for c in range(NCORES):
        b, hg = divmod(c, 4)
        e0 = hg * E
        cols = np.r_[e0:e0 + E, D + e0:D + e0 + E, 2 * D + e0:2 * D + e0 + E]
        wqkv_ = W_attn[:, cols].astype(BF16_NP)          # [D, 768]
        bq = b_attn[e0:e0 + E]
        bk = b_attn[D + e0:D + e0 + E]
        bv_ = b_attn[2 * D + e0:2 * D + e0 + E]
        bqk_ = np.concatenate([bq, bk]).reshape(4, 128).T.copy()  # [128, 4]
        wp_ = W_proj[e0:e0 + E, :].reshape(2, 128, D).transpose(
            1, 0, 2).reshape(128, 2 * D).astype(BF16_NP)
        in_maps.append({
            "xt": x[b].T.astype(BF16_NP),
            "wqkv": wqkv_,
            "bqk": bqk_,
            "bv": bv_.reshape(1, E).astype(BF16_NP),
            "ert": ert2,
            "wp": wp_,
            "ident": ident,
        })

    res = run_bass_kernel_spmd(nc, in_maps, core_ids=list(range(NCORES)),
                               trace=TRACE)
    _CACHE["last_results"] = res

    out = np.zeros((B, L, D), dtype=np.float32)
    for c in range(NCORES):
        out[c // 4] += res.results[c]["outt"].T.astype(np.float32)
    out += b_proj[None, None, :]
    return out
